# revision 31
# baseline (speedup 1.0000x reference)
"""BTV loss kernel for Trainium2 (8 NeuronCores, Bass/Tile).

reference: total = sum over 7x7 neighborhood shifts (k,l) != (0,0) of
           sqrt((x - roll(x,(k,l),axis=(2,3)))**2 + 1e-6).sum()
           out = 0.1 * total / x.size

Math: circular-shift symmetry halves the 48 shifts to 24 (doubled at
the end); sqrt(d^2+1e-6) ~= |d| (rel err ~3e-6); bf16 inputs add ~1e-5.

Layout (host-prepared, "band-major"): partition p holds rows
8p..8p+10 of each image (8 data bands + 3 halo bands), columns padded
circularly [4 left | 1024 | 3 right | 1 pad] = 1032.  A vertical shift
k is a band offset, a horizontal shift l a column offset -- both plain
AP offsets, so one DVE instruction covers a whole (image, shift) unit
of 128x8x1024 elements.  An odd-phase copy (cols shifted by one)
keeps odd-l operands 4B-aligned for DVE 2x mode.

Engines (work units = 24 shifts x 3 images = 72 per core):
  - DVE: custom fused |a-b|+accumulate op at 2x (~0.53 ns/elem),
    one op per unit, chained via the blk3 accumulator flop.
  - PE+ACT: remaining units as 512-col chunks: psum = I@base + (-I)@
    shifted (two matmuls), ACT drains Abs(psum) with accum_out.
  - Final: DVE reduces the f32 stage; host sums partials in f64.

Distribution: data parallel over the 24 (b,c) images, 3 per core.
"""

import dataclasses
import re
from operator import add as _py_add

import numpy as np

import concourse.bass as bass
import concourse.bacc as bacc_mod
import concourse.mybir as mybir
from concourse import dve_ops as _dvo
from concourse.dve_spec import AluOp as _DveAluOp
from concourse.dve_spec import Bin, Spec, Src0, Src1
from concourse.tile import TileContext
from concourse.bass_utils import run_bass_kernel_spmd

B, C, H, W = 8, 3, 1024, 1024
NCORES = 8
IMGS = (B * C) // NCORES        # images per core = 3
P = 128                         # partitions
BANDS = 11                      # 8 data bands + 3 halo
BASE = 4                        # left col pad (even => 4B-aligned bf16)
WPAD = W + BASE + 3 + 1         # 1032
SHIFTS = [(k, l) for k in range(0, 4) for l in range(-3, 4) if (k > 0 or l > 0)]
assert len(SHIFTS) == 24

# (shift, img) units on the PE+ACT lane; the rest run on DVE.
PE_SHIFTS = [(1, 3), (1, -3), (2, 3), (2, -3), (3, 3), (3, -3), (0, 3)]
# extra units for load balance ((shift, img)); (0,1) img0 leads so the
# PE lane starts as soon as the first bands of image 0 land, while the
# DVE lane leads with (0,2) img0 in fine pieces (k=0: band 0 suffices).
PE_EXTRA = [((1, 2), 0), ((0, 1), 0), ((0, 1), 1)]
# split units: bands [0, nb_pe) go to PE, the rest to DVE
SPLITS = {((0, 1), 2): 6, ((1, 0), 2): 2}
CHUNK = 512
GRP = 4                         # psum chunks per ACT drain (4 banks)

WEIGHT = 0.1
F32 = mybir.dt.float32
BF16 = mybir.dt.bfloat16

_OPS_CACHE = None


def _mk_absdiff_uop(two_x: bool, kind: str):
    """One UopConfig for the absdiff-accumulate family.

    kind: "seed"   - first element(-pair) loads the accumulator (blk3)
          "steady" - accumulates into blk3's CURR_ALU_OUT flop
    """
    from concourse.dve_uop import (
        ENABLE,
        AluInp,
        DelayInp,
        InpSel,
        OutPath,
        OutSel,
        Trigger,
        UopConfig,
    )
    from concourse.dve_uop import AluOp as UAluOp

    seed = kind == "seed"
    u = UopConfig()
    u.enable_input(InpSel.SRC_0, 0)
    u.enable_input(InpSel.SRC_1, 1)
    if two_x:
        u.enable_input(InpSel.SRC_0_HI, 2)
        u.enable_input(InpSel.SRC_1_HI, 3)
    u.accum_enabled = ENABLE
    dp = u.datapath_config
    dp[0].enable_alu(UAluOp.ABSOLUTE_DIFF, AluInp.PREV_ALU_OUT, AluInp.PREV_DELAY_0)
    if two_x:
        dp[0].pass_through_delay(1, 2)
        dp[1].enable_alu(
            UAluOp.ABSOLUTE_DIFF, AluInp.PREV_DELAY_1, AluInp.PREV_DELAY_2
        )
        dp[1].enable_delay_from_src(DelayInp.PREV_ALU_OUT, 0)
        dp[2].enable_alu(UAluOp.ADD, AluInp.PREV_ALU_OUT, AluInp.PREV_DELAY_0)
        dp[2].enable_delay_from_src(DelayInp.PREV_ALU_OUT, 1)
        dp[2].pass_through_delay(0)
    else:
        dp[1].enable_alu(UAluOp.BYPASS, AluInp.PREV_ALU_OUT, AluInp.PREV_ALU_OUT)
        dp[1].enable_delay_from_src(DelayInp.PREV_ALU_OUT, 0)
        dp[2].enable_alu(UAluOp.BYPASS, AluInp.PREV_ALU_OUT, AluInp.PREV_ALU_OUT)
        dp[2].pass_through_delay(0)
    if seed:
        dp[3].enable_alu(UAluOp.BYPASS, AluInp.PREV_ALU_OUT, AluInp.PREV_ALU_OUT)
    else:
        dp[3].enable_alu(UAluOp.ADD, AluInp.CURR_ALU_OUT, AluInp.PREV_ALU_OUT)
    dp[3].pass_through_delay(0, 1) if two_x else dp[3].pass_through_delay(0)
    dp[3].alu_out_a_enable = ENABLE
    for b in (4, 5, 6, 7):
        dp[b].enable_alu(UAluOp.BYPASS, AluInp.PREV_ALU_OUT, AluInp.PREV_ALU_OUT)
        dp[b].pass_through_delay(0, 1) if two_x else dp[b].pass_through_delay(0)
        dp[b].alu_out_a_enable = ENABLE
    u.require_inp0 = ENABLE
    u.require_inp1 = ENABLE
    u.enable_output(OutSel.DELAY_0, OutPath.WR0_LO)
    if two_x:
        u.enable_output(OutSel.DELAY_1, OutPath.WR0_HI)
    if seed:
        u.trigger = (Trigger.COUNT, Trigger.SRC_TENSOR_DONE, Trigger.NONE)
        u.next_uop = (1, 0, 0)
        u.repeat_count = 1
    else:
        u.trigger = (Trigger.SRC_TENSOR_DONE, Trigger.NONE, Trigger.NONE)
        u.next_uop = (0, 0, 0)
    return u


def _mk_read_uop():
    """Route blk3's persistent accumulator flop to WR0_LO."""
    from concourse.dve_uop import (
        ENABLE,
        AluInp,
        InpSel,
        OutPath,
        OutSel,
        Trigger,
        UopConfig,
    )
    from concourse.dve_uop import AluOp as UAluOp

    u = UopConfig()
    u.enable_input(InpSel.SRC_0, 0)
    dp = u.datapath_config
    for b in (0, 1, 2):
        dp[b].enable_alu(UAluOp.BYPASS, AluInp.PREV_ALU_OUT, AluInp.PREV_ALU_OUT)
    dp[3].enable_alu(UAluOp.BYPASS, AluInp.CURR_ALU_OUT, AluInp.CURR_ALU_OUT)
    for b in (4, 5, 6, 7):
        dp[b].enable_alu(UAluOp.BYPASS, AluInp.PREV_ALU_OUT, AluInp.PREV_ALU_OUT)
    u.require_inp0 = ENABLE
    u.enable_output(OutSel.ALU_OUT, OutPath.WR0_LO)
    u.trigger = (Trigger.SRC_TENSOR_DONE, Trigger.NONE, Trigger.NONE)
    u.next_uop = (0, 0, 0)
    return u


class _HandDveOp(_dvo.DveOp):
    """DveOp with hand-authored uop programs (1x and optional 2x)."""

    BUILDERS = {}  # name -> (build_1x, build_2x_or_None, rd1_en)

    def compile(self, ver):
        from concourse.dve_uop import DveOpSpec

        key = (self.name, ver)
        if (r := _dvo._COMPILE_CACHE.get(key)) is not None:
            return r
        b1, b2, rd1 = self.BUILDERS[self.name]
        result = DveOpSpec(
            name=self.name,
            opcode=_dvo.get_dve_sub_opcode(self.name),
            uops=b1(),
            uops_2x=(b2() if b2 is not None else None),
            rd1_en=rd1,
        )
        got = result.sha(ver)
        if self.uops_sha.get(ver) != got:
            raise ValueError(f"sha drift ({ver}: {got} != pinned)")
        _dvo._COMPILE_CACHE[key] = result
        return result


def _register(name, spec, build_1x, build_2x, rd1_en):
    _HandDveOp.BUILDERS[name] = (build_1x, build_2x, rd1_en)
    op = _HandDveOp(name, spec, subdim=False, uops_sha={})
    _dvo._SUB_OPCODE_FOR_NAME[name] = _dvo._CUSTOM_DVE_ROW_BASE + len(_dvo.OPS)
    shas = {}
    for ver in ("v3", "v4"):
        try:
            op.compile(ver)
            shas[ver] = op.uops_sha.get(ver)
        except ValueError as e:
            m = re.search(r"([0-9a-f]{16})", str(e))
            if not m:
                raise
            shas[ver] = m.group(1)
    op = dataclasses.replace(op, uops_sha=shas)
    _dvo.OPS.append(op)
    _dvo.CUSTOM_DVE_SPECS[name] = spec
    return op


def _get_ops():
    """Register (once per process) the custom DVE ops; return
    (seed, cont, read)."""
    global _OPS_CACHE
    if _OPS_CACHE is not None:
        return _OPS_CACHE
    have = {op.name: op for op in _dvo.OPS}
    if "ABSDIFF_ACC_SEED_ANT" in have:
        _OPS_CACHE = (
            have["ABSDIFF_ACC_SEED_ANT"],
            have["ABSDIFF_ACC_CONT_ANT"],
            have["ABSDIFF_ACC_READ_ANT"],
        )
        return _OPS_CACHE

    def _ref_acc(in0, in1, s0, s1, imm2):
        b = np.abs(in0.astype(np.float32) - in1.astype(np.float32)).astype(
            np.float32
        )
        return b, b.reshape(b.shape[0], -1).sum(axis=-1, keepdims=True)

    spec_acc = Spec(
        body=Bin(_DveAluOp.ABSOLUTE_DIFF, Src0, Src1),
        accum=_py_add,
        reference=_ref_acc,
    )
    spec_read = Spec(
        body=Src0,
        reference=lambda in0, in1, s0, s1, imm2: in0.astype(np.float32),
    )
    seed = _register(
        "ABSDIFF_ACC_SEED_ANT",
        spec_acc,
        lambda: [_mk_absdiff_uop(False, "seed"), _mk_absdiff_uop(False, "steady")],
        lambda: [_mk_absdiff_uop(True, "seed"), _mk_absdiff_uop(True, "steady")],
        True,
    )
    cont = _register(
        "ABSDIFF_ACC_CONT_ANT",
        spec_acc,
        lambda: [_mk_absdiff_uop(False, "steady")],
        lambda: [_mk_absdiff_uop(True, "steady")],
        True,
    )
    read = _register(
        "ABSDIFF_ACC_READ_ANT",
        spec_read,
        lambda: [_mk_read_uop()],
        None,
        False,
    )
    _OPS_CACHE = (seed, cont, read)
    return _OPS_CACHE


def _unit_assignment():
    """Return (dve_units, pe_units): lists of (shift_idx, img, b0, nb),
    image-major, low-k first (bands arrive in DMA order)."""
    pe = set()
    for s in PE_SHIFTS:
        si = SHIFTS.index(s)
        for i in range(IMGS):
            pe.add((si, i))
    for s, i in PE_EXTRA:
        pe.add((SHIFTS.index(s), i))
    splits = {
        (SHIFTS.index(s), i): nb_pe for (s, i), nb_pe in SPLITS.items()
    }
    dve, peu = [], []
    for i in range(IMGS):
        order = sorted(range(len(SHIFTS)), key=lambda s: SHIFTS[s][0])
        for si in order:
            u = (si, i)
            if u in splits:
                nb_pe = splits[u]
                peu.append((si, i, 0, nb_pe))
                dve.append((si, i, nb_pe, 8 - nb_pe))
            elif u in pe:
                peu.append((si, i, 0, 8))
            else:
                dve.append((si, i, 0, 8))
    # image 0's first two DVE units start in fine pieces so compute
    # begins as soon as the first DMA chunks land (first unit is k=0,
    # whose first piece needs only band 0).
    first, second = dve[0], dve[1]
    dve[0:2] = [
        (first[0], 0, 0, 1), (first[0], 0, 1, 1),
        (first[0], 0, 2, 2), (first[0], 0, 4, 2), (first[0], 0, 6, 2),
        (second[0], 0, 0, 2), (second[0], 0, 2, 2),
        (second[0], 0, 4, 4),
    ]
    # img0: low k first (DMA chunk arrival); last image: high k first so
    # the lane's tail ends on the small split units.
    peu.sort(
        key=lambda u: (
            u[1],
            SHIFTS[u[0]][0] if u[1] < IMGS - 1 else -SHIFTS[u[0]][0],
        )
    )
    return dve, peu


def _build_nc():
    seed_op, cont_op, read_op = _get_ops()
    dve_units, pe_units = _unit_assignment()
    n_drains = sum(nb * (W // CHUNK) // GRP for _, _, _, nb in pe_units)
    nstage = IMGS + n_drains  # 1 col per DVE image-chain + 1 per drain

    nc = bacc_mod.Bacc("TRN2", target_bir_lowering=False)
    X = nc.dram_tensor(
        "x", [IMGS, P, BANDS, WPAD], BF16, kind="ExternalInput"
    )
    WT = nc.dram_tensor("w", [P, 2 * P], BF16, kind="ExternalInput")
    OUT = nc.dram_tensor("out", [P, 1], F32, kind="ExternalOutput")

    with TileContext(nc) as tc:
        with (
            tc.tile_pool(name="data", bufs=1) as data_pool,
            tc.tile_pool(name="sc", bufs=1) as sc_pool,
            tc.tile_pool(name="acts", bufs=2) as acts_pool,
            tc.tile_pool(name="ps", bufs=2, space="PSUM") as ps_pool,
        ):
            wt = data_pool.tile([P, 2 * P], BF16)
            nc.sync.dma_start(out=wt[:], in_=WT[:])
            e = [
                data_pool.tile([P, BANDS, WPAD], BF16, name=f"e{i}")
                for i in range(IMGS)
            ]
            # DMA rings: only SP/Activation have hardware descriptor
            # generation (the Pool ring SW-generates at ~220ns/descriptor,
            # 32 GB/s -- never use it for bulk).  Each image is split
            # across both rings; image-major order so compute starts as
            # soon as e[0] lands.  (Unaligned odd-l operands run at full
            # 2x on the custom DVE op -- no odd-phase copy needed.)
            # image 0 lands in fine-grained chunks, low bands first, so
            # the first 1-band DVE pieces start ~8us in; halo bands 8-10
            # right behind; later images split across both rings.
            nc.sync.dma_start(out=e[0][:, 0:1], in_=X[0, :, 0:1])
            nc.scalar.dma_start(out=e[0][:, 1:2], in_=X[0, :, 1:2])
            nc.sync.dma_start(out=e[0][:, 2:4], in_=X[0, :, 2:4])
            nc.scalar.dma_start(out=e[0][:, 4:6], in_=X[0, :, 4:6])
            nc.sync.dma_start(out=e[0][:, 6:8], in_=X[0, :, 6:8])
            nc.scalar.dma_start(out=e[0][:, 8:], in_=X[0, :, 8:])
            HB = BANDS // 2
            for i in (1, 2):
                nc.scalar.dma_start(out=e[i][:, :HB], in_=X[i, :, :HB])
                nc.sync.dma_start(out=e[i][:, HB:], in_=X[i, :, HB:])
            scratch = sc_pool.tile([P, 8 * W], BF16)
            stage = sc_pool.tile([P, nstage], F32)
            wI = wt[:, 0:P]
            wnI = wt[:, P : 2 * P]

            def in1_ap(i, k, l, c0, nb=8, b0=0, width=None):
                wd = CHUNK if width is None else width
                return e[i][
                    :, b0 + k : b0 + k + nb, BASE + l + c0 : BASE + l + c0 + wd
                ]

            # ---- DVE lane: one chain per image ----
            col = 0
            by_img = {}
            for si, i, b0, nb in dve_units:
                by_img.setdefault(i, []).append((si, b0, nb))
            dve_chains = []
            for i in sorted(by_img):
                dve_chains.append((i, by_img[i], col))
                col += 1

            # ---- emit: interleave by image for early start ----
            # No tile_critical: chain contiguity on DVE is already forced
            # by WAW/WAR hazards on `scratch` (every op writes it, READ
            # reads it), and nothing else runs on the Vector engine.
            # tile_critical would drain ALL engines at each chain end,
            # serializing the PE/ACT lane against the DVE lane.
            for i, sis, scol in dve_chains:
                for j, (si, b0, nb) in enumerate(sis):
                    k, l = SHIFTS[si]
                    bi = nc.vector._custom_dve(
                        seed_op if j == 0 else cont_op,
                        out=scratch[:, 0 : nb * W],
                        in0=e[i][:, b0 : b0 + nb, BASE : BASE + W],
                        in1=in1_ap(i, k, l, 0, nb=nb, b0=b0, width=W),
                    )
                    bi.ins.perf_max = 1
                nc.vector._custom_dve(
                    read_op,
                    out=stage[:, scol : scol + 1],
                    in0=scratch[:, 0:1],
                )

            # ---- PE + ACT lane ----
            for ui, (si, i, ub0, nb) in enumerate(pe_units):
                k, l = SHIFTS[si]
                # nb bands x 2 col-chunks; groups of 4
                for g in range(nb * (W // CHUNK) // GRP):
                    psum = ps_pool.tile([P, GRP * CHUNK], F32, tag="ps")
                    for c in range(GRP):
                        chunk = g * GRP + c
                        b = ub0 + chunk % nb
                        c0 = (chunk // nb) * CHUNK
                        nc.tensor.matmul(
                            out=psum[:, c * CHUNK : (c + 1) * CHUNK],
                            lhsT=wI,
                            rhs=e[i][:, b, BASE + c0 : BASE + c0 + CHUNK],
                            start=True,
                            stop=False,
                        )
                        nc.tensor.matmul(
                            out=psum[:, c * CHUNK : (c + 1) * CHUNK],
                            lhsT=wnI,
                            rhs=in1_ap(i, k, l, c0, nb=1, b0=b)[:, 0, :],
                            start=False,
                            stop=True,
                        )
                    asc = acts_pool.tile([P, GRP * CHUNK], BF16, tag="asc")
                    nc.scalar.activation(
                        out=asc[:],
                        in_=psum[:],
                        func=mybir.ActivationFunctionType.Abs,
                        accum_out=stage[:, col : col + 1],
                    )
                    col += 1

            part = sc_pool.tile([P, 1], F32)
            nc.vector.tensor_reduce(
                out=part[:],
                in_=stage[:],
                axis=mybir.AxisListType.X,
                op=mybir.AluOpType.add,
            )
            nc.sync.dma_start(out=OUT[:], in_=part[:])
    return nc


_NC = None


def _get_nc():
    global _NC
    if _NC is None:
        _NC = _build_nc()
        if not _NC.is_finalized():
            _NC.finalize()
    return _NC


def _to_bf16(a32: np.ndarray) -> np.ndarray:
    b = np.ascontiguousarray(a32, dtype=np.float32).view(np.uint32)
    return ((b + 0x7FFF + ((b >> 16) & 1)) >> 16).astype(np.uint16)


def _prep_shards(x: np.ndarray) -> list[dict[str, np.ndarray]]:
    """bf16-cast and pack into the per-core band-major layout
    [IMGS, 2 phases, 128, BANDS, WPAD] (uint16 views of bf16)."""
    import ml_dtypes

    imgs = _to_bf16(x.reshape(B * C, H, W))  # (24, 1024, 1024) u16
    # circular column pad: [W-4..W-1][0..W-1][0..2][0]
    pad = np.zeros((B * C, H, WPAD), dtype=np.uint16)
    pad[:, :, BASE : BASE + W] = imgs
    pad[:, :, :BASE] = imgs[:, :, W - BASE :]
    pad[:, :, BASE + W : BASE + W + 3] = imgs[:, :, :3]
    # bands: E[p, b] = row (8p + b) % H
    rows = (8 * np.arange(P)[:, None] + np.arange(BANDS)[None, :]) % H
    even = pad[:, rows, :]  # (24, 128, 11, 1032)

    wk = np.zeros((P, 2 * P), dtype=np.float32)
    wk[:, 0:P] = np.eye(P)
    wk[:, P : 2 * P] = -np.eye(P)
    wv = wk.astype(ml_dtypes.bfloat16)

    out = []
    for ci in range(NCORES):
        xs = np.ascontiguousarray(even[ci * IMGS : (ci + 1) * IMGS])
        out.append({"x": xs.view(ml_dtypes.bfloat16), "w": wv})
    return out


def _run(x: np.ndarray, trace: bool = False):
    nc = _get_nc()
    in_maps = _prep_shards(x)
    res = run_bass_kernel_spmd(
        nc, in_maps, core_ids=list(range(NCORES)), trace=trace
    )
    total = 0.0
    for r in res.results:
        total += r["out"].astype(np.float64).sum()
    val = WEIGHT * 2.0 * total / float(B * C * H * W)
    return np.float32(val), res


def kernel(x: np.ndarray) -> np.ndarray:
    x = np.asarray(x, dtype=np.float32)
    val, _ = _run(x, trace=False)
    return val
